# revision 58
# baseline (speedup 1.0000x reference)
"""Trainium2 Bass kernel for ToRA-adapted windowed attention block.

Math (per batch image, S=1024 tokens, dim=768, 12 heads x 64):
  qkv  = x @ (Wqkv + U1 Gt U2^T)^T + b          Gt = G . U3[task]
  q,k,v split; attn = softmax(q k^T / 8) v ; out = attn-merge
  y    = out @ (Wp + U1p Gtp U2p^T)^T + bp

Strategy: data-parallel over B=8 - one image per NeuronCore, no
collectives.  Single-pass softmax per head:
  - host pre-computes effective weights (tiny low-rank update),
    pre-transposes them, folds the 1/8 softmax scale into Wq/bq, and
    reorders qk weight columns into per-head-pair blocks so the first
    pair can stream in ahead of the rest.
  - QKV runs feature-major for Q,K (Q^T/K^T tiles land directly on
    SBUF partitions, bias added during the Pool-engine evacuation) and
    token-major for V (evacuated to bf16 with the bias add fused,
    scattered into 65-wide per-head slots whose last column is ones so
    the A^T @ V_aug matmul also yields softmax denominators).
  - scores are computed ONCE, q-major: the row max comes from a single
    DVE reduce over the two-bank PSUM tile (negate=True makes it
    directly the exp bias; hardware allows only one PSUM input per
    vector op, so the cheaper fused two-input reduce is not legal),
    exp runs on ACT with the per-partition bias and writes bf16, and
    the DMA XBAR transposes A to k-major for the A^T @ V contraction.
    No stats matmul pass, no margin - the max is exact.
  - issue order interleaves everything against the reduce/exp-paced
    pipeline: V-matmul blocks fill the first two heads' gaps, each
    QK weight pair is produced one head-pair ahead of use, and each
    head's A^T V fills the next head's score phase.
  - softmax 1/l: the attention output (with the ones-column giving l)
    is evacuated to SBUF by ACT, DVE takes the reciprocal row, PE
    broadcasts it rank-1 into PSUM, ACT evacuates the broadcast, and
    Pool (which may not touch PSUM) does the SBUF-only multiply into
    bf16 attnT. V / proj biases are folded into the matmuls as rank-1
    accumulation steps so evacuations stay off the critical engines.
All big matmuls run as float32r or fp16 in / FP32 accumulate
(~5.7e-3 rel err end to end on hardware).
"""

import os
import sys
from contextlib import ExitStack

import numpy as np

sys.path.insert(0, "/opt/trn_rl_repo")

import concourse.bass as bass
import concourse.tile as tile
from concourse import bacc, mybir
from concourse.bass_utils import run_bass_kernel_spmd

F32 = mybir.dt.float32
F16 = mybir.dt.float16
F32R = mybir.dt.float32r
BF16 = mybir.dt.bfloat16
AX = mybir.AxisListType.X
OP = mybir.AluOpType
EXP = mybir.ActivationFunctionType.Exp
IDENT = mybir.ActivationFunctionType.Identity

D = 768          # model dim
KT = 6           # contract tiles over D
S = 1024         # tokens per image
NH = 12
HD = 64

N_CORES = 8


def build_program():
    nc = bacc.Bacc(
        "TRN2",
        target_bir_lowering=False,
        debug=False,
        enable_asserts=True,
        num_devices=N_CORES,
    )
    xT = nc.dram_tensor("xT", [D, S], F16, kind="ExternalInput").ap()
    # qk weights, columns reordered into 256-wide head-pair blocks
    # [Wq'_p | Wk_p]; pair 0 ships separately so it can load first.
    # fp16 inputs halve the startup DMA stream (verified ~4.6e-3 rel err).
    WqkA = nc.dram_tensor("WqkA", [D, 256], F16, kind="ExternalInput").ap()
    WqkB = nc.dram_tensor("WqkB", [D, 5 * 256], F16, kind="ExternalInput").ap()
    WvT = nc.dram_tensor("WvT", [D, D], F16, kind="ExternalInput").ap()
    WpT = nc.dram_tensor("WpT", [D, D], BF16, kind="ExternalInput").ap()
    bqk = nc.dram_tensor("bqk", [128, 12], F32, kind="ExternalInput").ap()
    bvh = nc.dram_tensor("bvh", [D], F16, kind="ExternalInput").ap()
    bpb16 = nc.dram_tensor("bpb16", [D], BF16, kind="ExternalInput").ap()
    y = nc.dram_tensor("y", [S, D], F32, kind="ExternalOutput").ap()
    DBG = os.environ.get("K_DBG", "") == "1"
    SKIP = set(os.environ.get("K_SKIP", "").split(","))
    if DBG:
        dx = nc.dram_tensor("dx", [128, KT, S], F16, kind="ExternalOutput").ap()
        dwa = nc.dram_tensor("dwa", [128, KT, 256], F16, kind="ExternalOutput").ap()
        dqkt = nc.dram_tensor("dqkt", [128, 2, S], F32R, kind="ExternalOutput").ap()
        dat = nc.dram_tensor("dat", [128, 8, S], BF16, kind="ExternalOutput").ap()
        dvau = nc.dram_tensor("dvau", [128, 8, NH * 65], BF16, kind="ExternalOutput").ap()
        datt = nc.dram_tensor("datt", [128, KT, S], BF16, kind="ExternalOutput").ap()

    with tile.TileContext(nc) as tc:
      with ExitStack() as stk:
        persist = stk.enter_context(tc.tile_pool(name="persist", bufs=1))
        Vaug = persist.tile([128, 8, NH * 65], BF16)  # V tok-major + ones col
        attnT = persist.tile([128, KT, S], BF16)      # attn out^T, normalized
        WpT_sb = persist.tile([128, KT, D], BF16)
        bqk_sb = persist.tile([128, 12], F32)
        bv_row = persist.tile([1, D], F16)
        bp_row = persist.tile([1, D], BF16)
        ones1 = persist.tile([1, 128], F32R)
        ones1h = persist.tile([1, 128], F16)
        ones1b = persist.tile([1, 128], BF16)

        # persistent small inputs go on the ACT hwdge queue so they do
        # not delay the x / weight stream on SP
        if "brow" not in SKIP:
            nc.scalar.dma_start(out=bv_row, in_=bvh.rearrange("(o d) -> o d", o=1))
            nc.scalar.dma_start(out=bp_row, in_=bpb16.rearrange("(o d) -> o d", o=1))
        nc.scalar.dma_start(out=bqk_sb, in_=bqk)
        nc.vector.memset(ones1.bitcast(mybir.dt.uint32), 0x3F800000)
        if "o16" not in SKIP:
            nc.vector.memset(ones1h.bitcast(mybir.dt.uint16), 0x3C00)
            nc.vector.memset(ones1b.bitcast(mybir.dt.uint16), 0x3F80)
        # whole-tile memset: V evac overwrites cols 0..63 of each head
        # slot, col 64 stays 1.0 (the denominator column)
        if "vmem" in SKIP:
            nc.vector.memset(Vaug.bitcast(mybir.dt.uint32), 0x3F803F80)
        else:
            nc.gpsimd.memset(Vaug, 1.0)
        if "attn" in SKIP or "p2" in SKIP or "rec" in SKIP:
            nc.gpsimd.memset(attnT, 0.0)

        PAD = int(os.environ.get("K_PAD", "0"))
        if PAD:
            padp = stk.enter_context(tc.tile_pool(name="padp", bufs=1))
            padt = padp.tile([128, PAD * 256], F32)
            nc.vector.memset(padt[:, 0:1], 0.0)
        qkvw = stk.enter_context(tc.tile_pool(name="qkvw", bufs=1))
        qktp = stk.enter_context(tc.tile_pool(name="qktp", bufs=3))
        aqp = stk.enter_context(tc.tile_pool(name="aqp", bufs=4))
        atp = stk.enter_context(tc.tile_pool(name="atp", bufs=3))
        statp = stk.enter_context(tc.tile_pool(name="stat", bufs=4))
        ysump = stk.enter_context(tc.tile_pool(name="ysum", bufs=1))
        psosb = stk.enter_context(tc.tile_pool(name="psosb", bufs=3))
        recp = stk.enter_context(tc.tile_pool(name="recp", bufs=2))
        sps = stk.enter_context(tc.tile_pool(name="sps", bufs=3, space="PSUM"))
        rbcp = stk.enter_context(tc.tile_pool(name="rbcp", bufs=2))
        psop = stk.enter_context(tc.tile_pool(name="pso", bufs=1, space="PSUM"))
        psbp = stk.enter_context(tc.tile_pool(name="psb", bufs=1, space="PSUM"))
        # wvtp is entered LAST so it can be released (LIFO) once V is done,
        # freeing its SBUF for the proj-output pool
        wvt_stk = ExitStack()
        wvtp = wvt_stk.enter_context(tc.tile_pool(name="wvtp", bufs=1))

        xT_sb = qkvw.tile([128, KT, S], F16)
        WqkA_sb = qkvw.tile([128, KT, 256], F16)
        WqkB_sb = qkvw.tile([128, KT, 5 * 256], F16)
        WvT_sb = wvtp.tile([128, KT, D], F16)

        # input stream on SP queue: pair-0 weights, then x, then V
        # weights, then the remaining qk pairs
        WqkA_r = WqkA.rearrange("(k p) f -> p k f", p=128)
        xT_r = xT.rearrange("(k p) t -> p k t", p=128)
        WvT_r = WvT.rearrange("(k p) f -> p k f", p=128)
        WqkB_r = WqkB.rearrange("(k p) f -> p k f", p=128)
        for kt in range(KT):
            nc.sync.dma_start(out=WqkA_sb[:, kt, :], in_=WqkA_r[:, kt, :])
            nc.sync.dma_start(out=xT_sb[:, kt, :], in_=xT_r[:, kt, :])
        for g in range(2):
            nc.sync.dma_start(
                out=WvT_sb[:, 3 * g:3 * g + 3, :], in_=WvT_r[:, 3 * g:3 * g + 3, :]
            )
        for g in range(2):
            nc.sync.dma_start(
                out=WqkB_sb[:, 3 * g:3 * g + 3, :],
                in_=WqkB_r[:, 3 * g:3 * g + 3, :],
            )
        if "wpt" not in SKIP:
            nc.scalar.dma_start(
                out=WpT_sb, in_=WpT.rearrange("(k p) f -> p k f", p=128)
            )

        # ---------------- work blocks ----------------
        def ft_alloc(j):
            return qktp.tile([128, 2, S], F32R, tag="qkt", name=f"qkt{j}")

        def ft_chunk(j, QKTj, half, qc, ps_hold):
            """One quarter of head pair j's QKV: 6 matmuls; evac on qc==1.
            ps_hold is a 1-element list carrying the psum tile across the
            two qc chunks of a half."""
            src = WqkA_sb if j == 0 else WqkB_sb
            base = 0 if j == 0 else (j - 1) * 256
            ft = j if half == 0 else 6 + j
            if qc == 0:
                ps_hold[0] = sps.tile([128, 1024], F32, tag="psS",
                                      name=f"psqk{j}_{half}")
            ps = ps_hold[0]
            for kt in range(KT):
                nc.tensor.matmul(
                    ps[:, qc * 512:(qc + 1) * 512],
                    src[:, kt, base + half * 128:base + half * 128 + 128],
                    xT_sb[:, kt, qc * 512:(qc + 1) * 512],
                    start=(kt == 0),
                    stop=(kt == KT - 1),
                )
            if qc == 1:
                if half == 0:
                    nc.scalar.activation(
                        QKTj[:, half, :], ps, IDENT,
                        bias=bqk_sb[:, ft:ft + 1], scale=1.0,
                    )
                else:
                    nc.vector.tensor_scalar(
                        out=QKTj[:, half, :], in0=ps,
                        scalar1=bqk_sb[:, ft:ft + 1], scalar2=None, op0=OP.add,
                    )

        def ft_pair(j):
            """QKV for head pair j issued as one serial block."""
            QKTj = ft_alloc(j)
            hold = [None]
            for half in (0, 1):
                for qc in (0, 1):
                    ft_chunk(j, QKTj, half, qc, hold)
            return QKTj

        def v_tt(tt):
            """V for token tile tt; bias folded in as a rank-1 PE step,
            evacuated to bf16 head slots by ACT."""
            ps = sps.tile([128, 1024], F32, tag="psS")
            for f0, fl in ((0, 512), (512, 256)):
                for kt in range(KT):
                    nc.tensor.matmul(
                        ps[:, f0:f0 + fl],
                        xT_sb[:, kt, tt * 128:(tt + 1) * 128],
                        WvT_sb[:, kt, f0:f0 + fl],
                        start=(kt == 0),
                        stop=(kt == KT - 1 and "vbias" in SKIP),
                    )
                if "vbias" not in SKIP:
                    nc.tensor.matmul(
                        ps[:, f0:f0 + fl], ones1h, bv_row[:, f0:f0 + fl],
                        start=False, stop=True,
                    )
            vdst = bass.AP(
                tensor=Vaug.tensor,
                offset=Vaug.offset + tt * NH * 65,
                ap=[[Vaug.ap[0][0], 128], [65, NH], [1, 64]],
            )
            if "vev" in SKIP:
                nc.vector.tensor_copy(
                    Vaug[:, tt, 0:768], ps[:, 0:768],
                )
            else:
                nc.scalar.activation(
                    vdst, ps[:, 0:768].rearrange("p (h d) -> p h d", d=64),
                    IDENT,
                )

        def phase1(h, QKTj, fills):
            """Scores + softmax + transpose for head h; fills is a list
            of (qt_slot, closure) PE filler work injected between score
            tiles."""
            off = (h % 2) * 64
            if "attn" in SKIP:
                if "fills" not in SKIP:
                    for qt in range(8):
                        for slot, fn in fills:
                            if slot == qt:
                                fn()
                return None
            AT = atp.tile([128, 8, S], BF16, tag="AT")
            for qt in range(8):
                psS = sps.tile([128, 1024], F32, tag="psS")
                for kc in range(2):
                    nc.tensor.matmul(
                        psS[:, kc * 512:(kc + 1) * 512],
                        QKTj[off:off + 64, 0, qt * 128:(qt + 1) * 128],
                        QKTj[off:off + 64, 1, kc * 512:(kc + 1) * 512],
                        start=True,
                        stop=True,
                    )
                mn = statp.tile([128, 1], F32, tag="mn")
                if os.environ.get("K_NEG", "1") == "1":
                    nc.vector.tensor_reduce(
                        out=mn, in_=psS, axis=AX, op=OP.max, negate=True,
                    )
                else:
                    mx = statp.tile([128, 1], F32, tag="mx")
                    nc.vector.tensor_reduce(
                        out=mx, in_=psS, axis=AX, op=OP.max,
                    )
                    nc.vector.tensor_scalar(
                        out=mn, in0=mx, scalar1=-1.0, scalar2=None,
                        op0=OP.mult,
                    )
                aq = aqp.tile([128, 1024], BF16, tag="aq")
                nc.scalar.activation(aq, psS, EXP, bias=mn[:, 0:1], scale=1.0)
                if "tr" not in SKIP:
                    nc.sync.dma_start(
                        out=AT[:, :, qt * 128:(qt + 1) * 128], in_=aq,
                        transpose=True,
                    )
                for slot, fn in fills:
                    if slot == qt:
                        fn()
            return AT

        def phase2_qc(h, AT, qc, st):
            """A^T V contraction for one 512-token column block of head h.
            The softmax 1/l normalization is deferred: the reciprocal rows
            are stashed (st) and flushed per head via a Pool-SWDGE DRAM
            broadcast, entirely off the critical path."""
            fq, off = h // 2, (h % 2) * 64
            if "attn" in SKIP or "p2" in SKIP:
                return
            pso = psop.tile([65, 512], F32, tag="pso")
            for kt in range(8):
                nc.tensor.matmul(
                    pso,
                    Vaug[:, kt, h * 65:(h + 1) * 65],
                    AT[:, kt, qc * 512:(qc + 1) * 512],
                    start=(kt == 0),
                    stop=(kt == 7),
                )
            po = psosb.tile([65, 512], F32, tag="po", name=f"po{h}_{qc}")
            nc.scalar.activation(po, pso, IDENT)
            rec = recp.tile([1, 512], F32R, tag="rec")
            with nc.allow_low_precision(reason="softmax 1/l"):
                nc.vector.reciprocal(rec, po[64:65, :])
            if "rec" in SKIP:
                return
            psb = psbp.tile([64, 512], F32, tag="psb")
            nc.tensor.matmul(psb, ones1[:, 0:64], rec, start=True, stop=True)
            rbc = rbcp.tile([64, 512], F32, tag="rbc")
            nc.scalar.activation(rbc, psb, IDENT)
            nc.gpsimd.tensor_tensor(
                attnT[off:off + 64, fq, qc * 512:(qc + 1) * 512],
                po[0:64, :],
                rbc,
                OP.mult,
            )

        def proj_tt(tt):
            ysb = ysump.tile([128, D], F32, name=f"ysum{tt}")
            if "wpt" in SKIP or "brow" in SKIP:
                nc.vector.memset(ysb, 0.0)
                nc.scalar.dma_start(out=y[tt * 128:(tt + 1) * 128, :], in_=ysb)
                return
            psy = sps.tile([128, 1024], F32, tag="psS", name=f"psy{tt}")
            for f0, fl in ((0, 512), (512, 256)):
                for kt in range(KT):
                    nc.tensor.matmul(
                        psy[:, f0:f0 + fl],
                        attnT[:, kt, tt * 128:(tt + 1) * 128],
                        WpT_sb[:, kt, f0:f0 + fl],
                        start=(kt == 0),
                        stop=False,
                    )
                nc.tensor.matmul(
                    psy[:, f0:f0 + fl], ones1b, bp_row[:, f0:f0 + fl],
                    start=False, stop=True,
                )
            nc.vector.tensor_copy(ysb, psy[:, 0:768])
            nc.scalar.dma_start(out=y[tt * 128:(tt + 1) * 128, :], in_=ysb)

        # ---------------- schedule ----------------
        # head 0/1 run against the V blocks as PE filler (V weights land
        # after x, so V work only becomes ready partway into head 0);
        # from head 2 on, the previous head's A^T V is the filler and the
        # next weight pair's matmuls are spread through the odd head's
        # score phase so the exp pipeline is never starved.
        AThs = {}
        QKTjs = {}
        QKTjs[0] = ft_pair(0)
        if DBG:
            nc.scalar.dma_start(out=dqkt, in_=QKTjs[0])
            nc.scalar.dma_start(out=dx, in_=xT_sb)
            nc.scalar.dma_start(out=dwa, in_=WqkA_sb)

        fills0 = [(3, lambda: v_tt(0)), (4, lambda: v_tt(1)),
                  (5, lambda: v_tt(2)), (6, lambda: v_tt(3)),
                  (7, lambda: v_tt(4))]
        AThs[0] = phase1(0, QKTjs[0], fills0)
        FT15 = "ft15" in SKIP
        if not FT15:
            QKTjs[1] = ft_alloc(1)
        h1_hold = [None]
        fills1 = [(qt, lambda tt=tt: v_tt(tt))
                  for qt, tt in enumerate(range(5, 8))]
        if not FT15:
            fills1 += [
                (3, lambda: ft_chunk(1, QKTjs[1], 0, 0, h1_hold)),
                (4, lambda: ft_chunk(1, QKTjs[1], 0, 1, h1_hold)),
                (5, lambda: ft_chunk(1, QKTjs[1], 1, 0, h1_hold)),
                (6, lambda: ft_chunk(1, QKTjs[1], 1, 1, h1_hold)),
            ]
        AThs[1] = phase1(1, QKTjs[0], fills1)
        if FT15:
            for j in range(1, 6):
                QKTjs[j] = QKTjs[0]
        if DBG:
            if AThs[0] is not None:
                nc.scalar.dma_start(out=dat, in_=AThs[0])
            nc.scalar.dma_start(out=dvau, in_=Vaug)

        for h in range(2, NH):
            if h == 2:
                at0, at1 = AThs[0], AThs[1]
                st0, st1 = {}, {}
                fills = [(0, lambda: phase2_qc(0, at0, 0, st0)),
                         (2, lambda: phase2_qc(0, at0, 1, st0)),
                         (4, lambda: phase2_qc(1, at1, 0, st1)),
                         (6, lambda: phase2_qc(1, at1, 1, st1))]
            else:
                hp, atp_h, stp = h - 1, AThs[h - 1], {}
                fills = [(1, lambda: phase2_qc(hp, atp_h, 0, stp)),
                         (4, lambda: phase2_qc(hp, atp_h, 1, stp))]
            # proj partials become ready two heads after their pair
            # completes; spread them one half per head

            if h % 2 == 1 and h < NH - 1 and not FT15:
                # produce the next head pair's weights inside this head
                jn = (h + 1) // 2
                QKTjs[jn] = ft_alloc(jn)
                hold = [None]
                fills += [
                    (2, lambda: ft_chunk(jn, QKTjs[jn], 0, 0, hold)),
                    (3, lambda: ft_chunk(jn, QKTjs[jn], 0, 1, hold)),
                    (5, lambda: ft_chunk(jn, QKTjs[jn], 1, 0, hold)),
                    (6, lambda: ft_chunk(jn, QKTjs[jn], 1, 1, hold)),
                ]
            AThs[h] = phase1(h, QKTjs[h // 2], fills)
            if h == 2:
                wvt_stk.close()

        if DBG:
            nc.scalar.dma_start(out=datt, in_=attnT)
        st11 = {}
        phase2_qc(11, AThs[11], 0, st11)
        phase2_qc(11, AThs[11], 1, st11)
        for tt in range(8):
            proj_tt(tt)

    nc.compile()
    return nc


_NC = None


def _get_nc():
    global _NC
    if _NC is None:
        _NC = build_program()
    return _NC


def prep_inputs(x, qkv_w, qkv_b, U1_qkv, U2_qkv, U3_qkv, G_qkv,
                proj_w, proj_b, U1_p, U2_p, U3_p, G_p, task_idx):
    import ml_dtypes
    t = int(task_idx)
    f = np.float32
    x = np.asarray(x, f)
    qkv_w = np.asarray(qkv_w, f)
    qkv_b = np.asarray(qkv_b, f)
    proj_w = np.asarray(proj_w, f)
    proj_b = np.asarray(proj_b, f)

    Gt = np.einsum("pqv,v->pq", np.asarray(G_qkv, f), np.asarray(U3_qkv, f)[t])
    Wqkv = qkv_w + np.asarray(U1_qkv, f) @ Gt @ np.asarray(U2_qkv, f).T
    Gtp = np.einsum("pqv,v->pq", np.asarray(G_p, f), np.asarray(U3_p, f)[t])
    Wp = proj_w + np.asarray(U1_p, f) @ Gtp @ np.asarray(U2_p, f).T

    Wqkv[:D] *= 0.125          # fold softmax scale into Wq / bq
    bq = qkv_b.copy()
    bq[:D] *= 0.125

    WqkT = np.ascontiguousarray(Wqkv[: 2 * D].T)      # [768 in, 1536 out]
    # reorder into per-head-pair 256-col blocks [Wq_j | Wk_j]
    pair_blocks = [
        np.concatenate(
            [WqkT[:, j * 128:(j + 1) * 128], WqkT[:, D + j * 128:D + (j + 1) * 128]],
            axis=1,
        )
        for j in range(6)
    ]
    WqkA = np.ascontiguousarray(pair_blocks[0]).astype(np.float16)
    WqkB = np.ascontiguousarray(np.concatenate(pair_blocks[1:], axis=1)).astype(np.float16)
    WvT = np.ascontiguousarray(Wqkv[2 * D:].T).astype(np.float16)
    WpT = np.ascontiguousarray(Wp.T).astype(ml_dtypes.bfloat16)
    bqk = np.ascontiguousarray(bq[: 2 * D].reshape(12, 128).T)
    bvh = np.ascontiguousarray(qkv_b[2 * D:]).astype(np.float16)
    bpb16 = np.ascontiguousarray(proj_b).astype(ml_dtypes.bfloat16)

    B = x.shape[0]
    xr = x.reshape(B, S, D)
    in_maps = [
        dict(
            xT=np.ascontiguousarray(xr[c].T).astype(np.float16),
            WqkA=WqkA, WqkB=WqkB, WvT=WvT, WpT=WpT, bqk=bqk, bvh=bvh, bpb16=bpb16,
        )
        for c in range(B)
    ]
    return in_maps


def run(in_maps, trace=False):
    nc = _get_nc()
    res = run_bass_kernel_spmd(nc, in_maps, list(range(N_CORES)), trace=trace)
    return res


def kernel(x, **kw):
    B, H, W, C = x.shape
    in_maps = prep_inputs(x, **kw)
    res = run(in_maps)
    out = np.stack([np.asarray(res.results[c]["y"]) for c in range(B)])
    return out.reshape(B, H, W, C).astype(np.float32)
